# revision 25
# baseline (speedup 1.0000x reference)
"""Trainium2 Bass kernel for nn_CrossAttention_31078383354530.

Reference (b=2, n=m=2048, qd=1024, cd=768, heads=8, dh=128):
    q = x @ Wq; k = ctx @ Wk; v = ctx @ Wv  (8 heads of 128)
    sim over the FLATTENED (b*n)=4096 token axis (batches mix - reference
    replicates an einops bug); softmax((sim-mean)*1.5+mean) ==
    softmax(1.5*scale*sim) exactly; out = attn @ v; y = out @ Wout + bout.

Sharding (8 cores): HEAD-sharded tensor parallel. Every core gets the FULL
x^T / ctx^T plus only its head's Wq/Wk/Wv columns and Wout rows. Each core
computes its head's whole 4096x4096 attention locally (no K/V exchange at
all), projects its head's contribution to y^T per 512-token q-chunk, and a
per-chunk bf16 ReduceScatter(add) over qd-chunks sums the 8 heads while the
next chunk computes. Core c ends up with y^T rows [128c,128c+128) for all
4096 tokens; the host reassembles. bout/8 is folded into each partial.

Why this beats the 340us AllGather baseline:
  - microbench: the PE sustains 216ns per 512-col bf16 matmul (full 2.4GHz)
    for 100us+ under an 8-core storm, and keeps that p-state through short
    stalls. The old kernel's K/V AllGather chain (49us barrier + 8x20us)
    paced attention with long stalls that kept the PE at ~1.2GHz.
  - head sharding has identical per-core FLOPs but only exchanges the tiny
    1MB-per-chunk output partials, pipelined behind compute.
  - attention is paced by scalar-engine exp (~15.5us per 512-q chunk); the
    PE stream (sim groups + 4-group-lagged pv + out-proj partials) is
    emitted so it never waits on exp.
  - V is produced as V^T (512-col matmuls) then flipped to [ctx,dh] tiles
    by 32 DMA xbar transposes (no PE/DVE cost).
  - softmax denominators: DVE pairwise tree -> ones-matmul column sum ->
    reciprocal_approx_fast -> ones-broadcast matmul -> DVE multiply.
"""

import sys

if "/opt/trn_rl_repo" not in sys.path:
    sys.path.insert(0, "/opt/trn_rl_repo")

import ml_dtypes
import numpy as np

import concourse.bass as bass  # noqa: F401
import concourse.mybir as mybir
import concourse.tile as tile
from concourse import bacc, bass_utils

F32 = mybir.dt.float32
BF16 = mybir.dt.bfloat16
AF = mybir.ActivationFunctionType
ALU = mybir.AluOpType

P = 128
N_CORES = 8
HEADS = 8
DH = 128
TOK = 4096              # b*n flattened token axis
QD = 1024
CD = 768
KC = QD // P            # 8 qd chunks
CC = CD // P            # 6 cd chunks
JT = TOK // P           # 32 ctx j-tiles
QC = 8                  # q-chunks per core
QW = TOK // QC          # 512 q tokens per chunk
NG = 11                 # sim groups per chunk (3,3,...,3,2)
LAG = 4                 # pv trails sim by LAG positions
TAU_SCALE = 1.5 * (DH ** -0.5)

_CACHE = {}


def _build():
    nc = bacc.Bacc(num_devices=N_CORES)

    xT = nc.declare_dram_parameter("xT", [QD, TOK], BF16, isOutput=False)
    cT = nc.declare_dram_parameter("cT", [CD, TOK], BF16, isOutput=False)
    wq = nc.declare_dram_parameter("wq", [P, KC, DH], BF16, isOutput=False)
    wk = nc.declare_dram_parameter("wk", [P, CC, DH], BF16, isOutput=False)
    wv = nc.declare_dram_parameter("wv", [P, CC, DH], BF16, isOutput=False)
    wo = nc.declare_dram_parameter("wo", [P, QD], BF16, isOutput=False)
    ones16 = nc.declare_dram_parameter("ones16", [P, P], BF16, isOutput=False)
    ones32 = nc.declare_dram_parameter("ones32", [P, P], F32, isOutput=False)
    yT = nc.declare_dram_parameter("yT", [QC // 2, P, 2 * QW], BF16,
                                   isOutput=True)

    with tile.TileContext(nc) as tc:
        with (
            tc.tile_pool(name="const", bufs=1) as const,
            tc.tile_pool(name="sb", bufs=1) as sb,
            tc.tile_pool(name="ps", bufs=1, space="PSUM") as ps,
            tc.tile_pool(name="dram", bufs=1, space="DRAM") as dram,
        ):
            # ---------------- DRAM exchange buffers ----------------
            NP_ = QC // 2   # RS ops (2 q-chunks each)
            rs_in = [dram.tile([N_CORES, P, 2 * QW], BF16, name=f"rs_in{q}")
                     for q in range(NP_)]
            rs_out = [dram.tile([P, 2 * QW], BF16, name=f"rs_out{q}")
                      for q in range(NP_)]
            warm_in = dram.tile([N_CORES, P, 8], BF16, name="warm_in")
            warm_out = dram.tile([P, 8], BF16, name="warm_out")

            # ---------------- input loads: split across the sync and
            # vector HWDGE queues so both queues' DMA rings pull
            # concurrently (per-ring bw is only ~22GB/s)
            cts, xts = [], []
            for k in range(CC):
                t = sb.tile([P, TOK], BF16, name=f"cts{k}", tag="cts", bufs=CC)
                eng = nc.sync if k < 3 else nc.scalar
                eng.dma_start(t[:], cT[k * P:(k + 1) * P, :])
                cts.append(t)
            wk_sb = const.tile([P, CC, DH], BF16, name="wk_sb")
            nc.sync.dma_start(wk_sb[:], wk[:, :, :])
            wv_sb = const.tile([P, CC, DH], BF16, name="wv_sb")
            nc.scalar.dma_start(wv_sb[:], wv[:, :, :])
            for k in range(KC):
                t = sb.tile([P, TOK], BF16, name=f"xts{k}", tag="xts", bufs=KC)
                eng = nc.sync if k < 4 else nc.scalar
                eng.dma_start(t[:], xT[k * P:(k + 1) * P, :])
                xts.append(t)
            wq_sb = const.tile([P, KC, DH], BF16, name="wq_sb")
            nc.scalar.dma_start(wq_sb[:], wq[:, :, :])
            wo_sb = const.tile([P, QD], BF16, name="wo_sb")
            nc.sync.dma_start(wo_sb[:], wo[:, :])
            o16_sb = const.tile([P, P], BF16, name="o16_sb")
            nc.sync.dma_start(o16_sb[:], ones16[:, :])
            o32_sb = const.tile([P, P], F32, name="o32_sb")
            nc.sync.dma_start(o32_sb[:], ones32[:, :])

            # warm up the collective stream early so the first real
            # ReduceScatter doesn't pay the ~30us cold-start
            nc.sync.dma_start(warm_in[:, :, :], o16_sb[:, :64])
            nc.gpsimd.collective_compute(
                "ReduceScatter", ALU.add,
                replica_groups=[list(range(N_CORES))],
                ins=[warm_in.opt()], outs=[warm_out.opt()])

            # ---------------- persistent SBUF results --------------
            kh = [sb.tile([P, QW], BF16, name=f"kh{i}", tag="kh", bufs=8)
                  for i in range(8)]      # K^T  [dh, ctx] in 512-chunks
            vTs = [sb.tile([P, QW], BF16, name=f"vT{i}", tag="vT", bufs=8)
                   for i in range(8)]     # V^T  [dh, ctx]
            vsb = [sb.tile([P, QW], BF16, name=f"vs{i}", tag="vs", bufs=8)
                   for i in range(8)]     # V    [ctx, dh] 4 j-tiles per tile
            qsb = [sb.tile([P, QW], BF16, name=f"qs{i}", tag="qs", bufs=8)
                   for i in range(8)]     # Q^T  [dh, q]

            pj_ctr = [0]

            def proj_tile(dst, w_sb, mov_tiles, nk, chunks, mov_of):
                """One [128,1536] psum tile holding len(chunks) 512-wide
                accumulations (contract over nk 128-chunks); copy each to
                dst[chunk] on DVE."""
                pj_ctr[0] += 1
                pt = ps.tile([P, 3 * QW], F32, name=f"pj{pj_ctr[0]}",
                             tag="sim", bufs=2)
                for gi, c in enumerate(chunks):
                    for k in range(nk):
                        nc.tensor.matmul(pt[:, gi * QW:(gi + 1) * QW],
                                         w_sb[:, k],
                                         mov_of(mov_tiles, k, c),
                                         start=(k == 0), stop=(k == nk - 1))
                for gi, c in enumerate(chunks):
                    nc.vector.tensor_copy(dst[c][:], pt[:, gi * QW:(gi + 1) * QW])

            mov_ctx = lambda tiles, k, c: tiles[k][:, c * QW:(c + 1) * QW]

            def _vt_flip(i):
                # one DMA xbar transpose: vTs[i] [dh, 512ctx] -> vsb[i]
                # holding four [128ctx, 128dh] j-tiles side by side
                nc.sync.dma_start(
                    vsb[i][:, :].rearrange("p (m d) -> p m d", d=DH),
                    vTs[i][:, :], transpose=True)

            # K proj (all of it, first) + Q chunk 0-2
            proj_tile(kh, wk_sb, cts, CC, [0, 1, 2], mov_ctx)
            proj_tile(qsb, wq_sb, xts, KC, [0, 1, 2], mov_ctx)
            proj_tile(kh, wk_sb, cts, CC, [3, 4, 5], mov_ctx)
            proj_tile(kh, wk_sb, cts, CC, [6, 7], mov_ctx)
            # V^T chunks 0-2 early so vsb j0..11 exist for pv of chunk 0
            proj_tile(vTs, wv_sb, cts, CC, [0, 1, 2], mov_ctx)
            for i in range(3):
                _vt_flip(i)

            # remaining projection work, doled out one item per position
            fillers = []
            fillers.append(lambda: proj_tile(vTs, wv_sb, cts, CC, [3, 4, 5],
                                             mov_ctx))
            fillers.append(lambda: (_vt_flip(3), _vt_flip(4)))
            fillers.append(lambda: proj_tile(vTs, wv_sb, cts, CC, [6, 7],
                                             mov_ctx))
            fillers.append(lambda: (_vt_flip(5), _vt_flip(6)))
            fillers.append(lambda: _vt_flip(7))
            fillers.append(lambda: proj_tile(qsb, wq_sb, xts, KC, [3, 4, 5],
                                             mov_ctx))
            fillers.append(lambda: proj_tile(qsb, wq_sb, xts, KC, [6, 7],
                                             mov_ctx))

            # ---------------- attention position stream ------------
            # position u = qc*NG + g ; at u: sim(u), filler, pv(u-LAG),
            # chunk-qc extras at fixed offsets past the chunk's last sim.
            groups = [list(range(3 * g, min(3 * g + 3, JT)))
                      for g in range(NG)]

            sim_ctx = {}   # u -> dict for pending pv
            at_of = {}     # u -> at tile
            chunk = {}     # qc -> state dict

            def emit_sim(u):
                qc, g = divmod(u, NG)
                js = groups[g]
                w = len(js) * QW
                sim_ps = ps.tile([P, 3 * QW], F32, name=f"s{u}", tag="sim",
                                 bufs=2)
                for jj, j in enumerate(js):
                    nc.tensor.matmul(
                        sim_ps[:, jj * QW:(jj + 1) * QW],
                        kh[j // 4][:, (j % 4) * DH:(j % 4 + 1) * DH],
                        qsb[qc][:],
                        start=True, stop=True)
                at = sb.tile([P, 3 * QW], BF16, name=f"at{u}", tag="at",
                             bufs=LAG + 2)
                nc.scalar.activation(at[:, :w], sim_ps[:, :w], AF.Exp,
                                     scale=TAU_SCALE)
                at_of[u] = at
                # DVE tree: join pairs as they become available
                st = chunk.setdefault(qc, {"lvl": [], "pend": None})
                if g < 10:
                    if st["pend"] is None:
                        st["pend"] = at
                    else:
                        # level-1 pair adds on the otherwise-idle Pool
                        # engine; the running joins stay on DVE
                        tr = sb.tile([P, 3 * QW], BF16, name=f"tr{u}",
                                     tag="tr", bufs=4)
                        nc.gpsimd.tensor_tensor(tr[:], st["pend"][:], at[:],
                                                ALU.add)
                        st["pend"] = None
                        st["lvl"].append(tr)
                        while len(st["lvl"]) >= 2:
                            a = st["lvl"].pop(0)
                            b = st["lvl"].pop(0)
                            tr2 = sb.tile([P, 3 * QW], BF16, name=f"tr{u}b",
                                          tag="tr", bufs=4)
                            nc.vector.tensor_tensor(tr2[:], a[:], b[:],
                                                    ALU.add)
                            st["lvl"].append(tr2)

            def emit_pv(u):
                qc, g = divmod(u, NG)
                js = groups[g]
                st = chunk[qc]
                if g == 0:
                    st["pv"] = ps.tile([P, QW], F32, name=f"pv{qc}", tag="pv",
                                       bufs=2)
                at = at_of.pop(u)
                for jj, j in enumerate(js):
                    nc.tensor.matmul(st["pv"][:],
                                     vsb[j // 4][:, (j % 4) * DH:(j % 4 + 1) * DH],
                                     at[:, jj * QW:(jj + 1) * QW],
                                     start=(j == 0), stop=(j == JT - 1))

            def emit_colsum(qc):
                # fold tree root + last(2-wide) group -> rs_sum, then ones
                # matmul -> [1,512] column sums
                st = chunk[qc]
                root = st["lvl"].pop()
                assert not st["lvl"] and st["pend"] is None
                last = at_of[qc * NG + 10]  # pv pops it one position later
                f1 = sb.tile([P, QW], BF16, name=f"f1_{qc}", tag="f1", bufs=2)
                nc.vector.tensor_tensor(f1[:], root[:, :QW],
                                        root[:, QW:2 * QW], ALU.add)
                f2 = sb.tile([P, QW], BF16, name=f"f2_{qc}", tag="f2", bufs=2)
                nc.vector.tensor_tensor(f2[:], f1[:], root[:, 2 * QW:3 * QW],
                                        ALU.add)
                f3 = sb.tile([P, QW], BF16, name=f"f3_{qc}", tag="f3", bufs=2)
                nc.vector.tensor_tensor(f3[:], f2[:], last[:, :QW], ALU.add)
                rs_sum = sb.tile([P, QW], BF16, name=f"rs{qc}", tag="rss",
                                 bufs=2)
                nc.vector.tensor_tensor(rs_sum[:], f3[:], last[:, QW:2 * QW],
                                        ALU.add)
                cs = ps.tile([P, QW], F32, name=f"cs{qc}", tag="pv", bufs=2)
                nc.tensor.matmul(cs[:1, :], o16_sb[:, :1], rs_sum[:],
                                 start=True, stop=True)
                st["cs"] = cs
                recip = sb.tile([P, QW], F32, name=f"rcp{qc}", tag="rcp",
                                bufs=2)
                nc.vector.reciprocal_approx_fast(recip[:1, :], cs[:1, :])
                st["recip"] = recip

            def emit_bcast(qc):
                st = chunk[qc]
                den = ps.tile([P, 3 * QW], F32, name=f"den{qc}", tag="sim",
                              bufs=2)
                nc.tensor.matmul(den[:, :QW], o32_sb[:1, :],
                                 st["recip"][:1, :], start=True, stop=True)
                den_sb = sb.tile([P, QW], F32, name=f"dsb{qc}", tag="dsb",
                                 bufs=2)
                nc.vector.tensor_copy(den_sb[:], den[:, :QW])
                osb = sb.tile([P, QW], BF16, name=f"osb{qc}", tag="osb",
                              bufs=2)
                nc.vector.tensor_tensor(osb[:], st["pv"][:], den_sb[:],
                                        ALU.mult)
                st["osb"] = osb

            def emit_proj(qc, ccs):
                # partial y^T chunks for this head: p[cc] = wo[:,cc]^T @ osb
                st = chunk[qc]
                pt = ps.tile([P, 3 * QW], F32, name=f"pp{qc}_{ccs[0]}",
                             tag="sim", bufs=2)
                for gi, cc2 in enumerate(ccs):
                    nc.tensor.matmul(pt[:, gi * QW:(gi + 1) * QW],
                                     wo_sb[:, cc2 * DH:(cc2 + 1) * DH],
                                     st["osb"][:], start=True, stop=True)
                half = (qc % 2) * QW
                for gi, cc2 in enumerate(ccs):
                    pc = sb.tile([P, QW], BF16, name=f"pc{qc}_{cc2}",
                                 tag="pc", bufs=4)
                    nc.vector.tensor_copy(pc[:], pt[:, gi * QW:(gi + 1) * QW])
                    nc.sync.dma_start(rs_in[qc // 2][cc2, :, half:half + QW],
                                      pc[:])

            def emit_rs(pair):
                nc.gpsimd.collective_compute(
                    "ReduceScatter", ALU.add,
                    replica_groups=[list(range(N_CORES))],
                    ins=[rs_in[pair].opt()], outs=[rs_out[pair].opt()])

            NPOS = QC * NG
            extras = {}  # position -> list of thunks
            for qc in range(QC):
                end = qc * NG + (NG - 1)   # position of last sim of chunk
                extras.setdefault(end + 3, []).append(
                    lambda q=qc: emit_colsum(q))
                extras.setdefault(end + 4, []).append(
                    lambda q=qc: emit_bcast(q))
                extras.setdefault(end + 5, []).append(
                    lambda q=qc: emit_proj(q, [0, 1, 2]))
                extras.setdefault(end + 6, []).append(
                    lambda q=qc: emit_proj(q, [3, 4, 5]))
                extras.setdefault(end + 7, []).append(
                    lambda q=qc: emit_proj(q, [6, 7]))
                if qc % 2 == 1:
                    extras.setdefault(end + 8, []).append(
                        lambda q=qc: emit_rs(q // 2))

            fill_iter = iter(fillers)
            for u in range(NPOS + LAG + 11):
                if u < NPOS:
                    emit_sim(u)
                    nxt = next(fill_iter, None)
                    if nxt is not None:
                        nxt()
                if u - LAG >= 0 and u - LAG < NPOS:
                    emit_pv(u - LAG)
                for th in extras.get(u, []):
                    th()

            # ---------------- final output DMAs --------------------
            # on the scalar queue: its exp work is done by now, and the
            # tile scheduler won't hoist these ahead of ready exps, so
            # the RS-completion waits never block a busy queue
            for pair in range(NP_):
                nc.scalar.dma_start(yT.ap()[pair], rs_out[pair][:, :])

    nc.compile()
    return nc


def _get_nc():
    if "nc" not in _CACHE:
        _CACHE["nc"] = _build()
    return _CACHE["nc"]


def _bf16(a):
    return np.ascontiguousarray(
        np.asarray(a, np.float32).astype(ml_dtypes.bfloat16))


def _prep_in_maps(x, context, Wq, Wk, Wv, Wout, bout):
    xT = _bf16(np.asarray(x, np.float32).reshape(TOK, QD).T)
    cT = _bf16(np.asarray(context, np.float32).reshape(TOK, CD).T)
    Wq = np.asarray(Wq, np.float32)
    Wk = np.asarray(Wk, np.float32)
    Wv = np.asarray(Wv, np.float32)
    Wout = np.asarray(Wout, np.float32)
    o16 = np.ones((P, P), np.float32)
    in_maps = []
    for c in range(N_CORES):
        h = slice(c * DH, (c + 1) * DH)
        in_maps.append({
            "xT": xT, "cT": cT,
            "wq": _bf16(Wq[:, h].reshape(KC, P, DH).transpose(1, 0, 2)),
            "wk": _bf16(Wk[:, h].reshape(CC, P, DH).transpose(1, 0, 2)),
            "wv": _bf16(Wv[:, h].reshape(CC, P, DH).transpose(1, 0, 2)),
            "wo": _bf16(Wout[h, :]),
            "ones16": _bf16(o16),
            "ones32": np.ascontiguousarray(o16),
        })
    return in_maps


def _assemble(results, bout):
    y = np.empty((TOK, QD), dtype=np.float32)
    for c in range(N_CORES):
        # [QC//2, P, 2*QW] -> per-q-chunk [P, QW] blocks, transposed
        yt = np.asarray(results[c]["yT"], dtype=np.float32)
        yt = yt.reshape(QC // 2, P, 2, QW)
        for qc in range(QC):
            y[qc * QW:(qc + 1) * QW, c * P:(c + 1) * P] = \
                yt[qc // 2, :, qc % 2, :].T
    y += np.asarray(bout, np.float32)[None, :]
    return y.reshape(2, TOK // 2, QD)


def run(inputs, trace=False, **kw):
    nc = _get_nc()
    in_maps = _prep_in_maps(**inputs)
    res = bass_utils.run_bass_kernel_spmd(
        nc, in_maps, core_ids=list(range(N_CORES)), trace=trace, **kw)
    return _assemble(res.results, inputs["bout"]), res


def kernel(**inputs):
    out, _ = run(inputs, trace=False)
    return out


# revision 28
# speedup vs baseline: 1.2672x; 1.2672x over previous
"""Trainium2 Bass kernel for nn_CrossAttention_31078383354530.

Reference (b=2, n=m=2048, qd=1024, cd=768, heads=8, dh=128):
    q = x @ Wq; k = ctx @ Wk; v = ctx @ Wv  (8 heads of 128)
    sim over the FLATTENED (b*n)=4096 token axis (batches mix - reference
    replicates an einops bug); softmax((sim-mean)*1.5+mean) ==
    softmax(1.5*scale*sim) exactly; out = attn @ v; y = out @ Wout + bout.

Sharding (8 cores): HEAD-sharded tensor parallel. Every core gets the FULL
x^T / ctx^T plus only its head's Wq/Wk/Wv columns and Wout rows. Each core
computes its head's whole 4096x4096 attention locally (no K/V exchange at
all), projects its head's contribution to y^T per 512-token q-chunk, and a
per-chunk bf16 ReduceScatter(add) over qd-chunks sums the 8 heads while the
next chunk computes. Core c ends up with y^T rows [128c,128c+128) for all
4096 tokens; the host reassembles. bout/8 is folded into each partial.

Why this beats the 340us AllGather baseline:
  - microbench: the PE sustains 216ns per 512-col bf16 matmul (full 2.4GHz)
    for 100us+ under an 8-core storm, and keeps that p-state through short
    stalls. The old kernel's K/V AllGather chain (49us barrier + 8x20us)
    paced attention with long stalls that kept the PE at ~1.2GHz.
  - head sharding has identical per-core FLOPs but only exchanges the tiny
    1MB-per-chunk output partials, pipelined behind compute.
  - attention is paced by scalar-engine exp (~15.5us per 512-q chunk); the
    PE stream (sim groups + 4-group-lagged pv + out-proj partials) is
    emitted so it never waits on exp.
  - V is produced as V^T (512-col matmuls) then flipped to [ctx,dh] tiles
    by 32 DMA xbar transposes (no PE/DVE cost).
  - softmax denominators: DVE pairwise tree -> ones-matmul column sum ->
    reciprocal_approx_fast -> ones-broadcast matmul -> DVE multiply.
"""

import sys

if "/opt/trn_rl_repo" not in sys.path:
    sys.path.insert(0, "/opt/trn_rl_repo")

import ml_dtypes
import numpy as np

import concourse.bass as bass  # noqa: F401
import concourse.mybir as mybir
import concourse.tile as tile
from concourse import bacc, bass_utils

F32 = mybir.dt.float32
BF16 = mybir.dt.bfloat16
AF = mybir.ActivationFunctionType
ALU = mybir.AluOpType

P = 128
N_CORES = 8
HEADS = 8
DH = 128
TOK = 4096              # b*n flattened token axis
QD = 1024
CD = 768
KC = QD // P            # 8 qd chunks
CC = CD // P            # 6 cd chunks
JT = TOK // P           # 32 ctx j-tiles
QC = 8                  # q-chunks per core
QW = TOK // QC          # 512 q tokens per chunk
NG = 11                 # sim groups per chunk (3,3,...,3,2)
LAG = 4                 # pv trails sim by LAG positions
TAU_SCALE = 1.5 * (DH ** -0.5)

_CACHE = {}


def _build():
    nc = bacc.Bacc(num_devices=N_CORES)

    xT = nc.declare_dram_parameter("xT", [QD, TOK], BF16, isOutput=False)
    cT = nc.declare_dram_parameter("cT", [CD, TOK], BF16, isOutput=False)
    wq = nc.declare_dram_parameter("wq", [P, KC, DH], BF16, isOutput=False)
    wk = nc.declare_dram_parameter("wk", [P, CC, DH], BF16, isOutput=False)
    wv = nc.declare_dram_parameter("wv", [P, CC, DH], BF16, isOutput=False)
    wo = nc.declare_dram_parameter("wo", [P, QD], BF16, isOutput=False)
    ones16 = nc.declare_dram_parameter("ones16", [P, P], BF16, isOutput=False)
    ones32 = nc.declare_dram_parameter("ones32", [P, P], F32, isOutput=False)
    yT = nc.declare_dram_parameter("yT", [QC // 2, P, 2 * QW], BF16,
                                   isOutput=True)

    with tile.TileContext(nc) as tc:
        with (
            tc.tile_pool(name="const", bufs=1) as const,
            tc.tile_pool(name="sb", bufs=1) as sb,
            tc.tile_pool(name="ps", bufs=1, space="PSUM") as ps,
            tc.tile_pool(name="dram", bufs=1, space="DRAM") as dram,
        ):
            # ---------------- DRAM exchange buffers ----------------
            NP_ = QC // 2   # RS ops (2 q-chunks each)
            rs_in = [dram.tile([N_CORES, P, 2 * QW], BF16, name=f"rs_in{q}")
                     for q in range(NP_)]
            rs_out = [dram.tile([P, 2 * QW], BF16, name=f"rs_out{q}")
                      for q in range(NP_)]
            warm_in = dram.tile([N_CORES, P, 8], BF16, name="warm_in")
            warm_out = dram.tile([P, 8], BF16, name="warm_out")

            # ---------------- input loads --------------------------
            # column-piece order (1536-col pieces, 3 q/ctx chunks each)
            # so the first projection chunks can start after ~3MB of
            # traffic instead of the full 14MB; alternate sync/scalar
            # queues to double issue throughput.
            PIECES = [(0, 1536), (1536, 1536), (3072, 1024)]
            cts = [sb.tile([P, TOK], BF16, name=f"cts{k}", tag="cts", bufs=CC)
                   for k in range(CC)]
            xts = [sb.tile([P, TOK], BF16, name=f"xts{k}", tag="xts", bufs=KC)
                   for k in range(KC)]

            def load_piece(dst_tiles, src, pi):
                off, w = PIECES[pi]
                for k, t in enumerate(dst_tiles):
                    eng = nc.sync if k % 2 == 0 else nc.scalar
                    eng.dma_start(t[:, off:off + w],
                                  src[k * P:(k + 1) * P, off:off + w])

            load_piece(cts, cT, 0)
            wk_sb = const.tile([P, CC, DH], BF16, name="wk_sb")
            nc.sync.dma_start(wk_sb[:], wk[:, :, :])
            load_piece(xts, xT, 0)
            wq_sb = const.tile([P, KC, DH], BF16, name="wq_sb")
            nc.scalar.dma_start(wq_sb[:], wq[:, :, :])
            wv_sb = const.tile([P, CC, DH], BF16, name="wv_sb")
            nc.scalar.dma_start(wv_sb[:], wv[:, :, :])
            wo_sb = const.tile([P, QD], BF16, name="wo_sb")
            nc.sync.dma_start(wo_sb[:], wo[:, :])
            o16_sb = const.tile([P, P], BF16, name="o16_sb")
            nc.sync.dma_start(o16_sb[:], ones16[:, :])
            o32_sb = const.tile([P, P], F32, name="o32_sb")
            nc.sync.dma_start(o32_sb[:], ones32[:, :])
            # warm up the collective stream early so the first real
            # ReduceScatter doesn't pay the cold-start
            nc.sync.dma_start(warm_in[:, :, :], o16_sb[:, :64])
            nc.gpsimd.collective_compute(
                "ReduceScatter", ALU.add,
                replica_groups=[list(range(N_CORES))],
                ins=[warm_in.opt()], outs=[warm_out.opt()])
            load_piece(cts, cT, 1)
            load_piece(xts, xT, 1)
            load_piece(cts, cT, 2)
            load_piece(xts, xT, 2)

            # ---------------- persistent SBUF results --------------
            kh = [sb.tile([P, QW], BF16, name=f"kh{i}", tag="kh", bufs=8)
                  for i in range(8)]      # K^T  [dh, ctx] in 512-chunks
            vTs = [sb.tile([P, QW], BF16, name=f"vT{i}", tag="vT", bufs=8)
                   for i in range(8)]     # V^T  [dh, ctx]
            vsb = [sb.tile([P, QW], BF16, name=f"vs{i}", tag="vs", bufs=8)
                   for i in range(8)]     # V    [ctx, dh] 4 j-tiles per tile
            qsb = [sb.tile([P, QW], BF16, name=f"qs{i}", tag="qs", bufs=8)
                   for i in range(8)]     # Q^T  [dh, q]

            pj_ctr = [0]

            def proj_tile(dst, w_sb, mov_tiles, nk, chunks, mov_of):
                """One [128,1536] psum tile holding len(chunks) 512-wide
                accumulations (contract over nk 128-chunks); copy each to
                dst[chunk] on DVE."""
                pj_ctr[0] += 1
                pt = ps.tile([P, 3 * QW], F32, name=f"pj{pj_ctr[0]}",
                             tag="sim", bufs=2)
                for gi, c in enumerate(chunks):
                    for k in range(nk):
                        nc.tensor.matmul(pt[:, gi * QW:(gi + 1) * QW],
                                         w_sb[:, k],
                                         mov_of(mov_tiles, k, c),
                                         start=(k == 0), stop=(k == nk - 1))
                for gi, c in enumerate(chunks):
                    nc.vector.tensor_copy(dst[c][:], pt[:, gi * QW:(gi + 1) * QW])

            mov_ctx = lambda tiles, k, c: tiles[k][:, c * QW:(c + 1) * QW]

            def _vt_flip(i):
                # one DMA xbar transpose: vTs[i] [dh, 512ctx] -> vsb[i]
                # holding four [128ctx, 128dh] j-tiles side by side
                nc.sync.dma_start(
                    vsb[i][:, :].rearrange("p (m d) -> p m d", d=DH),
                    vTs[i][:, :], transpose=True)

            # K proj (all of it, first) + Q chunk 0-2
            proj_tile(kh, wk_sb, cts, CC, [0, 1, 2], mov_ctx)
            proj_tile(qsb, wq_sb, xts, KC, [0, 1, 2], mov_ctx)
            proj_tile(kh, wk_sb, cts, CC, [3, 4, 5], mov_ctx)
            proj_tile(kh, wk_sb, cts, CC, [6, 7], mov_ctx)
            # V^T chunks 0-2 early so vsb j0..11 exist for pv of chunk 0
            proj_tile(vTs, wv_sb, cts, CC, [0, 1, 2], mov_ctx)
            for i in range(3):
                _vt_flip(i)

            # remaining projection work, doled out one item per position
            fillers = []
            fillers.append(lambda: proj_tile(vTs, wv_sb, cts, CC, [3, 4, 5],
                                             mov_ctx))
            fillers.append(lambda: (_vt_flip(3), _vt_flip(4)))
            fillers.append(lambda: proj_tile(vTs, wv_sb, cts, CC, [6, 7],
                                             mov_ctx))
            fillers.append(lambda: (_vt_flip(5), _vt_flip(6)))
            fillers.append(lambda: _vt_flip(7))
            fillers.append(lambda: proj_tile(qsb, wq_sb, xts, KC, [3, 4, 5],
                                             mov_ctx))
            fillers.append(lambda: proj_tile(qsb, wq_sb, xts, KC, [6, 7],
                                             mov_ctx))

            # ---------------- attention position stream ------------
            # position u = qc*NG + g ; at u: sim(u), filler, pv(u-LAG),
            # chunk-qc extras at fixed offsets past the chunk's last sim.
            groups = [list(range(3 * g, min(3 * g + 3, JT)))
                      for g in range(NG)]

            sim_ctx = {}   # u -> dict for pending pv
            at_of = {}     # u -> at tile
            chunk = {}     # qc -> state dict

            def emit_sim(u):
                qc, g = divmod(u, NG)
                js = groups[g]
                w = len(js) * QW
                sim_ps = ps.tile([P, 3 * QW], F32, name=f"s{u}", tag="sim",
                                 bufs=2)
                for jj, j in enumerate(js):
                    nc.tensor.matmul(
                        sim_ps[:, jj * QW:(jj + 1) * QW],
                        kh[j // 4][:, (j % 4) * DH:(j % 4 + 1) * DH],
                        qsb[qc][:],
                        start=True, stop=True)
                at = sb.tile([P, 3 * QW], BF16, name=f"at{u}", tag="at",
                             bufs=LAG + 2)
                nc.scalar.activation(at[:, :w], sim_ps[:, :w], AF.Exp,
                                     scale=TAU_SCALE)
                at_of[u] = at
                # DVE tree: join pairs as they become available
                st = chunk.setdefault(qc, {"lvl": [], "pend": None})
                if g < 10:
                    if st["pend"] is None:
                        st["pend"] = at
                    else:
                        tr = sb.tile([P, 3 * QW], BF16, name=f"tr{u}",
                                     tag="tr", bufs=4)
                        nc.vector.tensor_tensor(tr[:], st["pend"][:], at[:],
                                                ALU.add)
                        st["pend"] = None
                        st["lvl"].append(tr)
                        while len(st["lvl"]) >= 2:
                            a = st["lvl"].pop(0)
                            b = st["lvl"].pop(0)
                            tr2 = sb.tile([P, 3 * QW], BF16, name=f"tr{u}b",
                                          tag="tr", bufs=4)
                            nc.vector.tensor_tensor(tr2[:], a[:], b[:],
                                                    ALU.add)
                            st["lvl"].append(tr2)

            def emit_pv(u):
                qc, g = divmod(u, NG)
                js = groups[g]
                st = chunk[qc]
                if g == 0:
                    st["pv"] = ps.tile([P, QW], F32, name=f"pv{qc}", tag="pv",
                                       bufs=2)
                at = at_of.pop(u)
                for jj, j in enumerate(js):
                    nc.tensor.matmul(st["pv"][:],
                                     vsb[j // 4][:, (j % 4) * DH:(j % 4 + 1) * DH],
                                     at[:, jj * QW:(jj + 1) * QW],
                                     start=(j == 0), stop=(j == JT - 1))

            def emit_colsum(qc):
                # fold tree root + last(2-wide) group -> rs_sum, then ones
                # matmul -> [1,512] column sums
                st = chunk[qc]
                root = st["lvl"].pop()
                assert not st["lvl"] and st["pend"] is None
                last = at_of[qc * NG + 10]  # pv pops it one position later
                f1 = sb.tile([P, QW], BF16, name=f"f1_{qc}", tag="f1", bufs=2)
                nc.vector.tensor_tensor(f1[:], root[:, :QW],
                                        root[:, QW:2 * QW], ALU.add)
                f2 = sb.tile([P, QW], BF16, name=f"f2_{qc}", tag="f2", bufs=2)
                nc.vector.tensor_tensor(f2[:], f1[:], root[:, 2 * QW:3 * QW],
                                        ALU.add)
                f3 = sb.tile([P, QW], BF16, name=f"f3_{qc}", tag="f3", bufs=2)
                nc.vector.tensor_tensor(f3[:], f2[:], last[:, :QW], ALU.add)
                rs_sum = sb.tile([P, QW], BF16, name=f"rs{qc}", tag="rss",
                                 bufs=2)
                nc.vector.tensor_tensor(rs_sum[:], f3[:], last[:, QW:2 * QW],
                                        ALU.add)
                cs = ps.tile([P, QW], F32, name=f"cs{qc}", tag="pv", bufs=2)
                nc.tensor.matmul(cs[:1, :], o16_sb[:, :1], rs_sum[:],
                                 start=True, stop=True)
                st["cs"] = cs
                recip = sb.tile([P, QW], F32, name=f"rcp{qc}", tag="rcp",
                                bufs=2)
                nc.vector.reciprocal_approx_fast(recip[:1, :], cs[:1, :])
                st["recip"] = recip

            def emit_bcast(qc):
                st = chunk[qc]
                den = ps.tile([P, 3 * QW], F32, name=f"den{qc}", tag="sim",
                              bufs=2)
                nc.tensor.matmul(den[:, :QW], o32_sb[:1, :],
                                 st["recip"][:1, :], start=True, stop=True)
                den_sb = sb.tile([P, QW], F32, name=f"dsb{qc}", tag="dsb",
                                 bufs=2)
                nc.vector.tensor_copy(den_sb[:], den[:, :QW])
                osb = sb.tile([P, QW], BF16, name=f"osb{qc}", tag="osb",
                              bufs=2)
                nc.vector.tensor_tensor(osb[:], st["pv"][:], den_sb[:],
                                        ALU.mult)
                st["osb"] = osb

            def emit_proj(qc, ccs):
                # partial y^T chunks for this head: p[cc] = wo[:,cc]^T @ osb
                st = chunk[qc]
                pt = ps.tile([P, 3 * QW], F32, name=f"pp{qc}_{ccs[0]}",
                             tag="sim", bufs=2)
                for gi, cc2 in enumerate(ccs):
                    nc.tensor.matmul(pt[:, gi * QW:(gi + 1) * QW],
                                     wo_sb[:, cc2 * DH:(cc2 + 1) * DH],
                                     st["osb"][:], start=True, stop=True)
                half = (qc % 2) * QW
                for gi, cc2 in enumerate(ccs):
                    pc = sb.tile([P, QW], BF16, name=f"pc{qc}_{cc2}",
                                 tag="pc", bufs=4)
                    nc.vector.tensor_copy(pc[:], pt[:, gi * QW:(gi + 1) * QW])
                    nc.sync.dma_start(rs_in[qc // 2][cc2, :, half:half + QW],
                                      pc[:])

            def emit_rs(pair):
                nc.gpsimd.collective_compute(
                    "ReduceScatter", ALU.add,
                    replica_groups=[list(range(N_CORES))],
                    ins=[rs_in[pair].opt()], outs=[rs_out[pair].opt()])

            NPOS = QC * NG
            extras = {}  # position -> list of thunks
            for qc in range(QC):
                end = qc * NG + (NG - 1)   # position of last sim of chunk
                extras.setdefault(end + 3, []).append(
                    lambda q=qc: emit_colsum(q))
                extras.setdefault(end + 4, []).append(
                    lambda q=qc: emit_bcast(q))
                extras.setdefault(end + 5, []).append(
                    lambda q=qc: emit_proj(q, [0, 1, 2]))
                extras.setdefault(end + 6, []).append(
                    lambda q=qc: emit_proj(q, [3, 4, 5]))
                extras.setdefault(end + 7, []).append(
                    lambda q=qc: emit_proj(q, [6, 7]))
                if qc % 2 == 1:
                    extras.setdefault(end + 8, []).append(
                        lambda q=qc: emit_rs(q // 2))

            fill_iter = iter(fillers)
            for u in range(NPOS + LAG + 11):
                if u < NPOS:
                    emit_sim(u)
                    nxt = next(fill_iter, None)
                    if nxt is not None:
                        nxt()
                if u - LAG >= 0 and u - LAG < NPOS:
                    emit_pv(u - LAG)
                for th in extras.get(u, []):
                    th()

            # ---------------- final output DMAs --------------------
            # on the scalar queue: its exp work is done by now, and the
            # tile scheduler won't hoist these ahead of ready exps, so
            # the RS-completion waits never block a busy queue
            for pair in range(NP_):
                nc.scalar.dma_start(yT.ap()[pair], rs_out[pair][:, :])

    nc.compile()
    return nc


def _get_nc():
    if "nc" not in _CACHE:
        _CACHE["nc"] = _build()
    return _CACHE["nc"]


def _bf16(a):
    return np.ascontiguousarray(
        np.asarray(a, np.float32).astype(ml_dtypes.bfloat16))


def _prep_in_maps(x, context, Wq, Wk, Wv, Wout, bout):
    xT = _bf16(np.asarray(x, np.float32).reshape(TOK, QD).T)
    cT = _bf16(np.asarray(context, np.float32).reshape(TOK, CD).T)
    Wq = np.asarray(Wq, np.float32)
    Wk = np.asarray(Wk, np.float32)
    Wv = np.asarray(Wv, np.float32)
    Wout = np.asarray(Wout, np.float32)
    o16 = np.ones((P, P), np.float32)
    in_maps = []
    for c in range(N_CORES):
        h = slice(c * DH, (c + 1) * DH)
        in_maps.append({
            "xT": xT, "cT": cT,
            "wq": _bf16(Wq[:, h].reshape(KC, P, DH).transpose(1, 0, 2)),
            "wk": _bf16(Wk[:, h].reshape(CC, P, DH).transpose(1, 0, 2)),
            "wv": _bf16(Wv[:, h].reshape(CC, P, DH).transpose(1, 0, 2)),
            "wo": _bf16(Wout[h, :]),
            "ones16": _bf16(o16),
            "ones32": np.ascontiguousarray(o16),
        })
    return in_maps


def _assemble(results, bout):
    y = np.empty((TOK, QD), dtype=np.float32)
    for c in range(N_CORES):
        # [QC//2, P, 2*QW] -> per-q-chunk [P, QW] blocks, transposed
        yt = np.asarray(results[c]["yT"], dtype=np.float32)
        yt = yt.reshape(QC // 2, P, 2, QW)
        for qc in range(QC):
            y[qc * QW:(qc + 1) * QW, c * P:(c + 1) * P] = \
                yt[qc // 2, :, qc % 2, :].T
    y += np.asarray(bout, np.float32)[None, :]
    return y.reshape(2, TOK // 2, QD)


def run(inputs, trace=False, **kw):
    nc = _get_nc()
    in_maps = _prep_in_maps(**inputs)
    res = bass_utils.run_bass_kernel_spmd(
        nc, in_maps, core_ids=list(range(N_CORES)), trace=trace, **kw)
    return _assemble(res.results, inputs["bout"]), res


def kernel(**inputs):
    out, _ = run(inputs, trace=False)
    return out


# revision 37
# speedup vs baseline: 1.4473x; 1.1421x over previous
"""Trainium2 Bass kernel for nn_CrossAttention_31078383354530.

Reference (b=2, n=m=2048, qd=1024, cd=768, heads=8, dh=128):
    q = x @ Wq; k = ctx @ Wk; v = ctx @ Wv  (8 heads of 128)
    sim over the FLATTENED (b*n)=4096 token axis (batches mix - reference
    replicates an einops bug); softmax((sim-mean)*1.5+mean) ==
    softmax(1.5*scale*sim) exactly; out = attn @ v; y = out @ Wout + bout.

Sharding (8 cores): HEAD-sharded tensor parallel. Every core gets the FULL
x^T / ctx^T plus only its head's Wq/Wk/Wv columns and Wout rows. Each core
computes its head's whole 4096x4096 attention locally (no K/V exchange at
all), projects its head's contribution to y^T per 512-token q-chunk, and a
per-chunk bf16 ReduceScatter(add) over qd-chunks sums the 8 heads while the
next chunk computes. Core c ends up with y^T rows [128c,128c+128) for all
4096 tokens; the host reassembles. bout/8 is folded into each partial.

Why this beats the 340us AllGather baseline:
  - microbench: the PE sustains 216ns per 512-col bf16 matmul (full 2.4GHz)
    for 100us+ under an 8-core storm, and keeps that p-state through short
    stalls. The old kernel's K/V AllGather chain (49us barrier + 8x20us)
    paced attention with long stalls that kept the PE at ~1.2GHz.
  - head sharding has identical per-core FLOPs but only exchanges the tiny
    1MB-per-chunk output partials, pipelined behind compute.
  - attention is paced by scalar-engine exp (~15.5us per 512-q chunk); the
    PE stream (sim groups + 4-group-lagged pv + out-proj partials) is
    emitted so it never waits on exp.
  - V is produced as V^T (512-col matmuls) then flipped to [ctx,dh] tiles
    by 32 DMA xbar transposes (no PE/DVE cost).
  - softmax denominators: DVE pairwise tree -> ones-matmul column sum ->
    reciprocal_approx_fast -> ones-broadcast matmul -> DVE multiply.
"""

import sys

if "/opt/trn_rl_repo" not in sys.path:
    sys.path.insert(0, "/opt/trn_rl_repo")

import ml_dtypes
import numpy as np

import concourse.bass as bass  # noqa: F401
import concourse.mybir as mybir
import concourse.tile as tile
from concourse import bacc, bass_utils

F32 = mybir.dt.float32
BF16 = mybir.dt.bfloat16
AF = mybir.ActivationFunctionType
ALU = mybir.AluOpType

P = 128
N_CORES = 8
HEADS = 8
DH = 128
TOK = 4096              # b*n flattened token axis
QD = 1024
CD = 768
KC = QD // P            # 8 qd chunks
CC = CD // P            # 6 cd chunks
JT = TOK // P           # 32 ctx j-tiles
QC = 8                  # q-chunks per core
QW = TOK // QC          # 512 q tokens per chunk
NG = 11                 # sim groups per chunk (3,3,...,3,2)
LAG = 4                 # pv trails sim by LAG positions
TAU_SCALE = 1.5 * (DH ** -0.5)

_CACHE = {}


def _build():
    nc = bacc.Bacc(num_devices=N_CORES)

    xT = nc.declare_dram_parameter("xT", [QD, TOK], BF16, isOutput=False)
    cT = nc.declare_dram_parameter("cT", [CD, TOK], BF16, isOutput=False)
    wq = nc.declare_dram_parameter("wq", [P, KC, DH], BF16, isOutput=False)
    wk = nc.declare_dram_parameter("wk", [P, CC, DH], BF16, isOutput=False)
    wv = nc.declare_dram_parameter("wv", [P, CC, DH], BF16, isOutput=False)
    wof = nc.declare_dram_parameter("wof", [QD, QD], BF16, isOutput=False)
    ones16 = nc.declare_dram_parameter("ones16", [P, P], BF16, isOutput=False)
    ones32 = nc.declare_dram_parameter("ones32", [P, P], F32, isOutput=False)
    yT = nc.declare_dram_parameter("yT", [KC, P, QW], BF16, isOutput=True)

    with tile.TileContext(nc) as tc:
        with (
            tc.tile_pool(name="const", bufs=1) as const,
            tc.tile_pool(name="sb", bufs=1) as sb,
            tc.tile_pool(name="ps", bufs=1, space="PSUM") as ps,
            tc.tile_pool(name="dram", bufs=1, space="DRAM") as dram,
        ):
            # ---------------- DRAM exchange buffers ----------------
            at_in = dram.tile([N_CORES, P, QW], BF16, name="at_in")
            at_out = dram.tile([N_CORES, P, QW], BF16, name="at_out")
            warm_in = dram.tile([N_CORES, P, 8], BF16, name="warm_in")
            warm_out = dram.tile([N_CORES, P, 8], BF16, name="warm_out")

            # ---------------- input loads --------------------------
            # column-piece order (1536-col pieces, 3 q/ctx chunks each)
            # so the first projection chunks can start after ~3MB of
            # traffic instead of the full 14MB; alternate sync/scalar
            # queues to double issue throughput.
            PIECES = [(0, 1536), (1536, 1536), (3072, 1024)]
            cts = [sb.tile([P, TOK], BF16, name=f"cts{k}", tag="cts", bufs=CC)
                   for k in range(CC)]
            xts = [sb.tile([P, TOK], BF16, name=f"xts{k}", tag="xts", bufs=KC)
                   for k in range(KC)]

            def load_piece(dst_tiles, src, pi):
                off, w = PIECES[pi]
                for k, t in enumerate(dst_tiles):
                    eng = nc.sync if k % 2 == 0 else nc.scalar
                    eng.dma_start(t[:, off:off + w],
                                  src[k * P:(k + 1) * P, off:off + w])

            wk_sb = const.tile([P, CC, DH], BF16, name="wk_sb")
            nc.scalar.dma_start(wk_sb[:], wk[:, :, :])
            wv_sb = const.tile([P, CC, DH], BF16, name="wv_sb")
            nc.scalar.dma_start(wv_sb[:], wv[:, :, :])
            # all of cT first: K proj (the attention-start gate) never
            # waits on xT traffic
            load_piece(cts, cT, 0)
            load_piece(cts, cT, 1)
            load_piece(cts, cT, 2)
            wq_sb = const.tile([P, KC, DH], BF16, name="wq_sb")
            nc.scalar.dma_start(wq_sb[:], wq[:, :, :])
            load_piece(xts, xT, 0)
            o16_sb = const.tile([P, P], BF16, name="o16_sb")
            nc.scalar.dma_start(o16_sb[:], ones16[:, :])
            o32_sb = const.tile([P, P], F32, name="o32_sb")
            nc.scalar.dma_start(o32_sb[:], ones32[:, :])
            # warm up the collective stream early so the final AllToAll
            # doesn't pay the cold-start
            nc.scalar.dma_start(warm_in[:, :, :], o16_sb[:, :64])
            nc.gpsimd.collective_compute(
                "AllToAll", ALU.bypass,
                replica_groups=[list(range(N_CORES))],
                ins=[warm_in.opt()], outs=[warm_out.opt()])
            load_piece(xts, xT, 1)
            load_piece(xts, xT, 2)
            # Wout tiles (only needed at the tail projection): wo_cc[cc]
            # [128, ic, dh] with partitions = inner rows within chunk ic
            wo_cc = []
            for cc in range(KC):
                t = const.tile([P, KC, DH], BF16, name=f"wo{cc}")
                eng = nc.sync if cc % 2 == 0 else nc.scalar
                eng.dma_start(
                    t[:],
                    wof.ap()[:, cc * DH:(cc + 1) * DH].rearrange(
                        "(k p) c -> p k c", p=P))
                wo_cc.append(t)

            # ---------------- persistent SBUF results --------------
            kh = [sb.tile([P, QW], BF16, name=f"kh{i}", tag="kh", bufs=8)
                  for i in range(8)]      # K^T  [dh, ctx] in 512-chunks
            vTs = [sb.tile([P, QW], BF16, name=f"vT{i}", tag="vT", bufs=8)
                   for i in range(8)]     # V^T  [dh, ctx]
            vsb = [sb.tile([P, QW], BF16, name=f"vs{i}", tag="vs", bufs=8)
                   for i in range(8)]     # V    [ctx, dh] 4 j-tiles per tile
            qsb = [sb.tile([P, QW], BF16, name=f"qs{i}", tag="qs", bufs=8)
                   for i in range(8)]     # Q^T  [dh, q]

            pj_ctr = [0]

            def proj_tile(dst, w_sb, mov_tiles, nk, chunks, mov_of):
                """One [128,1536] psum tile holding len(chunks) 512-wide
                accumulations (contract over nk 128-chunks); copy each to
                dst[chunk] on DVE."""
                pj_ctr[0] += 1
                pt = ps.tile([P, 3 * QW], F32, name=f"pj{pj_ctr[0]}",
                             tag="sim", bufs=2)
                for gi, c in enumerate(chunks):
                    for k in range(nk):
                        nc.tensor.matmul(pt[:, gi * QW:(gi + 1) * QW],
                                         w_sb[:, k],
                                         mov_of(mov_tiles, k, c),
                                         start=(k == 0), stop=(k == nk - 1))
                for gi, c in enumerate(chunks):
                    nc.vector.tensor_copy(dst[c][:], pt[:, gi * QW:(gi + 1) * QW])

            mov_ctx = lambda tiles, k, c: tiles[k][:, c * QW:(c + 1) * QW]

            def _vt_flip(i):
                # one DMA xbar transpose: vTs[i] [dh, 512ctx] -> vsb[i]
                # holding four [128ctx, 128dh] j-tiles side by side
                nc.sync.dma_start(
                    vsb[i][:, :].rearrange("p (m d) -> p m d", d=DH),
                    vTs[i][:, :], transpose=True)

            # K proj (all of it, first) + Q chunk 0-2
            proj_tile(kh, wk_sb, cts, CC, [0, 1, 2], mov_ctx)
            proj_tile(qsb, wq_sb, xts, KC, [0, 1, 2], mov_ctx)
            proj_tile(kh, wk_sb, cts, CC, [3, 4, 5], mov_ctx)
            proj_tile(kh, wk_sb, cts, CC, [6, 7], mov_ctx)
            # V^T chunks 0-2 early so vsb j0..11 exist for pv of chunk 0
            proj_tile(vTs, wv_sb, cts, CC, [0, 1, 2], mov_ctx)
            for i in range(3):
                _vt_flip(i)

            # remaining projection work, doled out one item per position
            fillers = []
            fillers.append(lambda: proj_tile(vTs, wv_sb, cts, CC, [3, 4, 5],
                                             mov_ctx))
            fillers.append(lambda: (_vt_flip(3), _vt_flip(4)))
            fillers.append(lambda: proj_tile(vTs, wv_sb, cts, CC, [6, 7],
                                             mov_ctx))
            fillers.append(lambda: (_vt_flip(5), _vt_flip(6)))
            fillers.append(lambda: _vt_flip(7))
            fillers.append(lambda: proj_tile(qsb, wq_sb, xts, KC, [3, 4, 5],
                                             mov_ctx))
            fillers.append(lambda: proj_tile(qsb, wq_sb, xts, KC, [6, 7],
                                             mov_ctx))

            # ---------------- attention position stream ------------
            # position u = qc*NG + g ; at u: sim(u), filler, pv(u-LAG),
            # chunk-qc extras at fixed offsets past the chunk's last sim.
            groups = [list(range(3 * g, min(3 * g + 3, JT)))
                      for g in range(NG)]

            sim_ctx = {}   # u -> dict for pending pv
            at_of = {}     # u -> at tile
            chunk = {}     # qc -> state dict

            def emit_sim(u):
                qc, g = divmod(u, NG)
                js = groups[g]
                w = len(js) * QW
                sim_ps = ps.tile([P, 3 * QW], F32, name=f"s{u}", tag="sim",
                                 bufs=2)
                for jj, j in enumerate(js):
                    nc.tensor.matmul(
                        sim_ps[:, jj * QW:(jj + 1) * QW],
                        kh[j // 4][:, (j % 4) * DH:(j % 4 + 1) * DH],
                        qsb[qc][:],
                        start=True, stop=True)
                at = sb.tile([P, 3 * QW], BF16, name=f"at{u}", tag="at",
                             bufs=LAG + 2)
                nc.scalar.activation(at[:, :w], sim_ps[:, :w], AF.Exp,
                                     scale=TAU_SCALE)
                at_of[u] = at
                # DVE tree: join pairs as they become available
                st = chunk.setdefault(qc, {"lvl": [], "pend": None})
                if g < 10:
                    if st["pend"] is None:
                        st["pend"] = at
                    else:
                        tr = sb.tile([P, 3 * QW], BF16, name=f"tr{u}",
                                     tag="tr", bufs=3)
                        nc.vector.tensor_tensor(tr[:], st["pend"][:], at[:],
                                                ALU.add)
                        st["pend"] = None
                        st["lvl"].append(tr)
                        while len(st["lvl"]) >= 2:
                            a = st["lvl"].pop(0)
                            b = st["lvl"].pop(0)
                            tr2 = sb.tile([P, 3 * QW], BF16, name=f"tr{u}b",
                                          tag="tr", bufs=3)
                            nc.vector.tensor_tensor(tr2[:], a[:], b[:],
                                                    ALU.add)
                            st["lvl"].append(tr2)

            def emit_pv(u):
                qc, g = divmod(u, NG)
                js = groups[g]
                st = chunk[qc]
                if g == 0:
                    st["pv"] = ps.tile([P, QW], F32, name=f"pv{qc}", tag="pv",
                                       bufs=2)
                at = at_of.pop(u)
                for jj, j in enumerate(js):
                    nc.tensor.matmul(st["pv"][:],
                                     vsb[j // 4][:, (j % 4) * DH:(j % 4 + 1) * DH],
                                     at[:, jj * QW:(jj + 1) * QW],
                                     start=(j == 0), stop=(j == JT - 1))

            def emit_colsum(qc):
                # fold tree root + last(2-wide) group -> rs_sum, then ones
                # matmul -> [1,512] column sums
                st = chunk[qc]
                root = st["lvl"].pop()
                assert not st["lvl"] and st["pend"] is None
                last = at_of[qc * NG + 10]  # pv pops it one position later
                f1 = sb.tile([P, QW], BF16, name=f"f1_{qc}", tag="f1", bufs=1)
                nc.vector.tensor_tensor(f1[:], root[:, :QW],
                                        root[:, QW:2 * QW], ALU.add)
                f2 = sb.tile([P, QW], BF16, name=f"f2_{qc}", tag="f2", bufs=1)
                nc.vector.tensor_tensor(f2[:], f1[:], root[:, 2 * QW:3 * QW],
                                        ALU.add)
                f3 = sb.tile([P, QW], BF16, name=f"f3_{qc}", tag="f3", bufs=1)
                nc.vector.tensor_tensor(f3[:], f2[:], last[:, :QW], ALU.add)
                rs_sum = sb.tile([P, QW], BF16, name=f"rs{qc}", tag="rss",
                                 bufs=1)
                nc.vector.tensor_tensor(rs_sum[:], f3[:], last[:, QW:2 * QW],
                                        ALU.add)
                cs = ps.tile([P, QW], F32, name=f"cs{qc}", tag="pv", bufs=2)
                nc.tensor.matmul(cs[:1, :], o16_sb[:, :1], rs_sum[:],
                                 start=True, stop=True)
                st["cs"] = cs
                recip = sb.tile([P, QW], F32, name=f"rcp{qc}", tag="rcp",
                                bufs=1)
                nc.vector.reciprocal_approx_fast(recip[:1, :], cs[:1, :])
                st["recip"] = recip

            def emit_norm(qc):
                st = chunk[qc]
                den = ps.tile([P, 3 * QW], F32, name=f"den{qc}", tag="sim",
                              bufs=2)
                nc.tensor.matmul(den[:, :QW], o32_sb[:1, :],
                                 st["recip"][:1, :], start=True, stop=True)
                den_sb = sb.tile([P, QW], F32, name=f"dsb{qc}", tag="dsb",
                                 bufs=1)
                nc.vector.tensor_copy(den_sb[:], den[:, :QW])
                osb = sb.tile([P, QW], BF16, name=f"osb{qc}", tag="osb",
                              bufs=2)
                nc.vector.tensor_tensor(osb[:], st["pv"][:], den_sb[:],
                                        ALU.mult)
                st["osb"] = osb

            def emit_ship(qc):
                # stash this chunk's normalized head-output for the final
                # AllToAll (slot qc goes to core qc)
                nc.sync.dma_start(at_in[qc], chunk[qc]["osb"][:])

            NPOS = QC * NG
            extras = {}  # position -> list of thunks
            for qc in range(QC):
                end = qc * NG + (NG - 1)   # position of last sim of chunk
                extras.setdefault(end + 3, []).append(
                    lambda q=qc: emit_colsum(q))
                extras.setdefault(end + 4, []).append(
                    lambda q=qc: emit_norm(q))
                extras.setdefault(end + 5, []).append(
                    lambda q=qc: emit_ship(q))

            fill_iter = iter(fillers)
            for u in range(NPOS + LAG + 11):
                if u < NPOS:
                    emit_sim(u)
                    nxt = next(fill_iter, None)
                    if nxt is not None:
                        nxt()
                if u - LAG >= 0 and u - LAG < NPOS:
                    emit_pv(u - LAG)
                for th in extras.get(u, []):
                    th()

            # ---------------- tail: AllToAll + local projection -----
            nc.gpsimd.collective_compute(
                "AllToAll", ALU.bypass,
                replica_groups=[list(range(N_CORES))],
                ins=[at_in.opt()], outs=[at_out.opt()])
            og = [sb.tile([P, QW], BF16, name=f"og{ic}", tag="vT", bufs=8)
                  for ic in range(HEADS)]
            for ic in range(HEADS):
                nc.sync.dma_start(og[ic][:], at_out[ic])
            for cc in range(KC):
                y_ps = ps.tile([P, QW], F32, name=f"y{cc}", tag="pv", bufs=2)
                for ic in range(HEADS):
                    nc.tensor.matmul(y_ps[:], wo_cc[cc][:, ic], og[ic][:],
                                     start=(ic == 0), stop=(ic == HEADS - 1))
                ysb = sb.tile([P, QW], BF16, name=f"ysb{cc}", tag="ysb",
                              bufs=2)
                nc.vector.tensor_copy(ysb[:], y_ps[:])
                nc.sync.dma_start(yT.ap()[cc], ysb[:])


    nc.compile()
    return nc


def _get_nc():
    if "nc" not in _CACHE:
        _CACHE["nc"] = _build()
    return _CACHE["nc"]


def _bf16(a):
    return np.ascontiguousarray(
        np.asarray(a, np.float32).astype(ml_dtypes.bfloat16))


def _prep_in_maps(x, context, Wq, Wk, Wv, Wout, bout):
    xT = _bf16(np.asarray(x, np.float32).reshape(TOK, QD).T)
    cT = _bf16(np.asarray(context, np.float32).reshape(TOK, CD).T)
    Wq = np.asarray(Wq, np.float32)
    Wk = np.asarray(Wk, np.float32)
    Wv = np.asarray(Wv, np.float32)
    Wout_bf = _bf16(Wout)
    o16_bf = _bf16(np.ones((P, P), np.float32))
    o32_f = np.ascontiguousarray(np.ones((P, P), np.float32))
    in_maps = []
    for c in range(N_CORES):
        h = slice(c * DH, (c + 1) * DH)
        in_maps.append({
            "xT": xT, "cT": cT,
            "wq": _bf16(Wq[:, h].reshape(KC, P, DH).transpose(1, 0, 2)),
            "wk": _bf16(Wk[:, h].reshape(CC, P, DH).transpose(1, 0, 2)),
            "wv": _bf16(Wv[:, h].reshape(CC, P, DH).transpose(1, 0, 2)),
            "wof": Wout_bf,
            "ones16": o16_bf,
            "ones32": o32_f,
        })
    return in_maps


def _assemble(results, bout):
    y = np.empty((TOK, QD), dtype=np.float32)
    for c in range(N_CORES):
        yt = np.asarray(results[c]["yT"], dtype=np.float32)  # [KC, P, QW]
        for cc in range(KC):
            y[c * QW:(c + 1) * QW, cc * P:(cc + 1) * P] = yt[cc].T
    y += np.asarray(bout, np.float32)[None, :]
    return y.reshape(2, TOK // 2, QD)


def run(inputs, trace=False, **kw):
    nc = _get_nc()
    in_maps = _prep_in_maps(**inputs)
    res = bass_utils.run_bass_kernel_spmd(
        nc, in_maps, core_ids=list(range(N_CORES)), trace=trace, **kw)
    return _assemble(res.results, inputs["bout"]), res


def kernel(**inputs):
    out, _ = run(inputs, trace=False)
    return out
